# revision 29
# baseline (speedup 1.0000x reference)
"""MoE feed-forward kernel for 8 Trainium2 NeuronCores.

Strategy (v2, bf16):
  - Router (tiny: x @ rW, top-2, softmax) runs on host in numpy.
  - Expert-parallel: core e owns routed expert e. Host gathers the tokens
    routed to expert e (padded to the global max capacity C), ships them
    pre-transposed as (D, C) in bf16; the device runs gelu(x@W1+b1) @ W2
    with the per-token gate weight folded in on-chip. Host scatter-adds.
  - Shared experts: sharded (expert s = core//4, hidden-quarter q = core%4).
    Each core computes its quarter of one shared expert over all tokens;
    host sums the 8 bf16 partials (0.5 mean factor folded into sW2).
  - All matmul operands bf16 (halves SBUF read pressure vs fp32r and all
    DMA traffic; rel-err ~3.4e-3, well under the 2e-2 gate).
  - Both phases software-pipeline GEMM1/GEMM2 one block apart so the PE
    never waits on the activation engine.
  - Routed GEMM2 accumulates K=4096 in 6 standing PSUM banks per
    384-token group (3 tok rows x 2 d-halves) - no DVE adds at all.
  - W2 (8.4MB bf16) is fully SBUF-resident, prefetched in the shared
    phase; W1 streams per 128-hidden block on two queues.
"""

import sys
import types

import numpy as np
import ml_dtypes

sys.path.insert(0, "/opt/trn_rl_repo")

import concourse.bass as bass  # noqa: E402
import concourse.mybir as mybir  # noqa: E402
import concourse.tile as tile  # noqa: E402
from concourse import bacc  # noqa: E402
from concourse.bass_utils import run_bass_kernel_spmd  # noqa: E402

F32 = mybir.dt.float32
BF16 = mybir.dt.bfloat16
GELU = mybir.ActivationFunctionType.Gelu

D = 1024      # d_model
H = 4096      # expert hidden
HQ = 1024     # shared-expert hidden slice per core (H / 4)
T = 4096      # tokens (2 * 2048)
E = 8         # routed experts
TOP_K = 2
NCORES = 8
NHB = H // 128  # 32 hidden blocks in routed phase

BF = ml_dtypes.bfloat16


def _install_ntff_hook():
    """Shim for the missing antenv.axon_hooks so trace=True can profile."""
    try:
        import antenv
        if "antenv.axon_hooks" in sys.modules:
            return
        mod = types.ModuleType("antenv.axon_hooks")
        mod._hook = None
        mod.set_axon_ntff_profile_hook = lambda h: setattr(mod, "_hook", h)
        mod.get_axon_ntff_profile_hook = lambda: mod._hook
        sys.modules["antenv.axon_hooks"] = mod
        antenv.axon_hooks = mod
        sys.path.insert(0, "/root/.axon_site/trn_agent_boot")
        import trn_boot
        hook = trn_boot._ntff_profile_via_ctypes("/opt/axon/libaxon_pjrt.so")
        mod.set_axon_ntff_profile_hook(hook)
    except Exception:
        pass


def _tok_groups(CR):
    """Split CR token rows into groups of <=3 rows (6 PSUM banks each)."""
    groups = []
    r = 0
    while r < CR:
        n = min(3, CR - r)
        groups.append((r, n))
        r += n
    return groups


_NC_CACHE = {}


def _build_nc(C):
    if C in _NC_CACHE:
        return _NC_CACHE[C]
    CR = C // 128
    tgs = _tok_groups(CR)

    nc = bacc.Bacc("TRN2", target_bir_lowering=False, debug=False,
                   enable_asserts=True, num_devices=NCORES)

    xeT = nc.dram_tensor("xeT", (D, C), BF16, kind="ExternalInput")
    # small per-partition tensors are uploaded pre-transposed ([128, n],
    # partition-major) so their DMAs are contiguous lines, not 4B gathers
    g_d = nc.dram_tensor("g", (128, CR), F32, kind="ExternalInput")
    W1e = nc.dram_tensor("W1e", (D, H), BF16, kind="ExternalInput")
    W2e = nc.dram_tensor("W2e", (H, D), BF16, kind="ExternalInput")
    b1e = nc.dram_tensor("b1e", (128, NHB), F32, kind="ExternalInput")
    xT = nc.dram_tensor("xT", (D, T), BF16, kind="ExternalInput")
    sW1q = nc.dram_tensor("sW1q", (D, HQ), BF16, kind="ExternalInput")
    sW2q = nc.dram_tensor("sW2q", (HQ, D), BF16, kind="ExternalInput")
    sb1q = nc.dram_tensor("sb1q", (128, 8), F32, kind="ExternalInput")
    yr = nc.dram_tensor("yr", (C, D), BF16, kind="ExternalOutput")
    ys = nc.dram_tensor("ys", (T, D), BF16, kind="ExternalOutput")

    xer = xeT.ap().rearrange("(a p) t -> p a t", p=128)
    W1r = W1e.ap().rearrange("(a p) h -> p a h", p=128)
    W2r = W2e.ap().rearrange("(a p) d -> p a d", p=128)
    sw1r = sW1q.ap().rearrange("(a p) h -> p a h", p=128)
    sw2r = sW2q.ap().rearrange("(a p) d -> p a d", p=128)
    xTr = xT.ap().rearrange("(a p) t -> p a t", p=128)
    ysr = ys.ap().rearrange("(a p) d -> p a d", p=128)
    yrr = yr.ap().rearrange("(a p) d -> p a d", p=128)

    with tile.TileContext(nc) as tc:
        # long-lived pool: routed-phase tensors prefetched during phase S
        with tc.tile_pool(name="pre", bufs=1) as pre:
          w2f = pre.tile([128, NHB, D], BF16)     # full W2, 64KB/part
          xe = pre.tile([128, 8, C], BF16)        # routed tokens
          gt = pre.tile([128, CR], F32)
          b1t = pre.tile([128, NHB], F32)
          wus = pre.tile([128, 128], BF16)        # PE warmup operands
          wum = pre.tile([128, 512], BF16)

          # ---------------- phase S: shared-expert slice over all tokens ----
          with tc.tile_pool(name="swp", bufs=1) as swp, \
             tc.tile_pool(name="sxp", bufs=3) as sxp, \
             tc.tile_pool(name="shp", bufs=26) as shp, \
             tc.tile_pool(name="syp", bufs=3) as syp, \
             tc.tile_pool(name="sph", bufs=2, space="PSUM") as sph, \
             tc.tile_pool(name="spy", bufs=4, space="PSUM") as spy:
            sw1 = swp.tile([128, 8, HQ], BF16)
            sw2 = swp.tile([128, 8, D], BF16)
            sb1t = swp.tile([128, 8], F32)

            # Startup critical path: first GEMM chain needs sw1 + xs[0].
            # Only sync/scalar/gpsimd can initiate DMAs; xs[0] leads the
            # gpsimd queue, sw1 splits across sync+scalar (1MB each). Bulk
            # prefetches are deferred (emitted after block 0's activations
            # on the scalar stream) so they can't steal HBM bandwidth from
            # the critical startup loads.
            # interleave sw1 h-chunks across sync/scalar so consecutive
            # h-tiles become ready in near arrival order
            nc.sync.dma_start(sw1[:, :, 0:128], sw1r[:, :, 0:128])
            nc.scalar.dma_start(sw1[:, :, 128:256], sw1r[:, :, 128:256])
            nc.sync.dma_start(sw1[:, :, 256:512], sw1r[:, :, 256:512])
            nc.scalar.dma_start(sw1[:, :, 512:1024], sw1r[:, :, 512:1024])
            nc.sync.dma_start(sb1t[:], sb1q.ap()[:])
            # sw2 rides sync behind the sw1 chunks; with the lag-2 pipeline
            # GEMM2 of block 0 doesn't start until ~44us, so one queue is
            # fine and gpsimd stays a pure xs stream
            nc.sync.dma_start(sw2[:], sw2r[:])
            # PE warmup: run throwaway matmuls while the critical DMAs land
            # so the tensor engine is out of its low p-state when real work
            # starts. Operands come from DVE memsets (no DMA dependency).
            nc.vector.memset(wus[:], 0)
            nc.vector.memset(wum[:], 0)
            for wi in range(16):
                pw = sph.tile([128, 512], F32, tag="ph")
                nc.tensor.matmul(pw[:], wus[:], wum[:], start=True, stop=True)

            NB = T // 512
            xs_t = [None] * NB
            hts_t = [None] * NB

            def s_g1(cb):
                xs = sxp.tile([128, 8, 512], BF16, tag="xs")
                # split the load so the first d-pair lands early (startup)
                for dp in range(4):
                    nc.gpsimd.dma_start(
                        xs[:, 2 * dp:2 * dp + 2, :],
                        xTr[:, 2 * dp:2 * dp + 2, cb * 512:(cb + 1) * 512])
                xs_t[cb] = xs
                hts = []
                for h in range(8):
                    ph = sph.tile([128, 512], F32, tag="ph")
                    for d in range(8):
                        nc.tensor.matmul(ph[:], sw1[:, d, h * 128:(h + 1) * 128],
                                         xs[:, d, :], start=(d == 0), stop=(d == 7))
                    ht = shp.tile([128, 512], BF16, tag="ht")
                    nc.scalar.activation(ht[:], ph[:], GELU, bias=sb1t[:, h:h + 1])
                    hts.append(ht)
                hts_t[cb] = hts

            def s_g2(cb):
                hts = hts_t[cb]
                for cs in range(4):
                    for dh in range(2):
                        py = spy.tile([128, 512], F32, tag="py")
                        for h in range(8):
                            nc.tensor.matmul(py[:], hts[h][:, cs * 128:(cs + 1) * 128],
                                             sw2[:, h, dh * 512:(dh + 1) * 512],
                                             start=(h == 0), stop=(h == 7))
                        yt = syp.tile([128, 512], BF16, tag="yt")
                        nc.vector.tensor_copy(yt[:], py[:])
                        nc.gpsimd.dma_start(ysr[:, cb * 4 + cs, dh * 512:(dh + 1) * 512], yt[:])

            # software pipeline: G2(cb) sits between G1(cb+1) and G1(cb+2)
            s_g1(0)
            # routed-phase bulk prefetch. The scheduler ignores program
            # order for independent instructions, so gate these big DMAs
            # behind block-0 compute with a real (WAW) dependency: a tiny
            # DVE copy into each destination tile that reads block 0's
            # first h-tile. Without this the 10.7MB of prefetch saturates
            # HBM right at startup and starves the critical loads.
            ht00 = hts_t[0][0]
            nc.vector.tensor_copy(xe[:, 0, 0:2], ht00[:, 0:2])
            nc.vector.tensor_copy(w2f[:, 0, 0:2], ht00[:, 0:2])
            nc.vector.tensor_copy(gt[:, 0:1], ht00[:, 0:1])
            nc.vector.tensor_copy(b1t[:, 0:1], ht00[:, 0:1])
            nc.scalar.dma_start(xe[:], xer[:])
            nc.scalar.dma_start(w2f[:], W2r[:])
            nc.sync.dma_start(gt[:], g_d.ap()[:])
            nc.sync.dma_start(b1t[:], b1e.ap()[:])
            # lag-2 software pipeline: G2(cb) between G1(cb+2) and G1(cb+3),
            # so block 0's GEMM2 never waits on the sw2 load
            s_g1(1)
            s_g1(2)
            s_g2(0)
            for cb in range(3, NB):
                s_g1(cb)
                s_g2(cb - 3 + 1)
            s_g2(NB - 2)
            s_g2(NB - 1)

          # ---------------- phase R: routed expert -------------------------
          with tc.tile_pool(name="rwp", bufs=8) as rwp, \
             tc.tile_pool(name="rhp", bufs=3) as rhp, \
             tc.tile_pool(name="rgp", bufs=3) as rgp, \
             tc.tile_pool(name="rph", bufs=2, space="PSUM") as rph, \
             tc.tile_pool(name="rac", bufs=1, space="PSUM") as rac:
            for gi, (r0, nr) in enumerate(tgs):
                c0, ct = r0 * 128, nr * 128
                accs = [rac.tile([128, 512], F32, tag=f"acc{i}", bufs=1,
                                 name=f"acc{i}") for i in range(2 * nr)]
                ht_prev = None

                def r_g2(hb, ht):
                    for tr in range(nr):
                        for dh in range(2):
                            acc = accs[tr * 2 + dh]
                            nc.tensor.matmul(acc[:], ht[:, tr * 128:(tr + 1) * 128],
                                             w2f[:, hb, dh * 512:(dh + 1) * 512],
                                             start=(hb == 0), stop=(hb == NHB - 1))
                            if hb == NHB - 1:
                                # gate + store now so the epilogue overlaps;
                                # alternate DVE/scalar so the final muls of
                                # the last group run two-wide
                                yg = rgp.tile([128, 512], BF16, tag="yg")
                                crow = r0 + tr
                                if (tr * 2 + dh) % 2 == 0:
                                    nc.vector.tensor_scalar_mul(
                                        yg[:], acc[:], gt[:, crow:crow + 1])
                                else:
                                    nc.scalar.mul(
                                        yg[:], acc[:], gt[:, crow:crow + 1])
                                q = nc.sync if dh == 0 else nc.gpsimd
                                q.dma_start(
                                    yrr[:, crow, dh * 512:(dh + 1) * 512], yg[:])

                for hb in range(NHB):
                    w1t = rwp.tile([128, 8, 128], BF16, tag="w1")
                    q = nc.sync if hb % 2 == 0 else nc.gpsimd
                    q.dma_start(w1t[:], W1r[:, :, hb * 128:(hb + 1) * 128])
                    w1 = w1t[:]
                    ph = rph.tile([128, 512], F32, tag="ph")
                    for d in range(8):
                        nc.tensor.matmul(ph[:, :ct], w1[:, d, :],
                                         xe[:, d, c0:c0 + ct],
                                         start=(d == 0), stop=(d == 7))
                    ht = rhp.tile([128, 512], BF16, tag="ht")
                    nc.scalar.activation(ht[:, :ct], ph[:, :ct], GELU,
                                         bias=b1t[:, hb:hb + 1])
                    if ht_prev is not None:
                        r_g2(hb - 1, ht_prev)
                    ht_prev = ht
                r_g2(NHB - 1, ht_prev)

    nc.compile()
    nc.finalize()
    _NC_CACHE[C] = nc
    return nc


def _route(xf, rW, rb):
    """Host router: replicates jax top_k (ties -> lower index) + softmax."""
    gates = xf @ rW + rb
    idx = np.argsort(-gates, axis=1, kind="stable")[:, :TOP_K]
    vals = np.take_along_axis(gates, idx, axis=1)
    ex = np.exp(vals - vals[:, :1])
    probs = (ex / ex.sum(axis=1, keepdims=True)).astype(np.float32)
    return idx, probs


def _run(inputs, trace=False):
    x = np.asarray(inputs["x"], dtype=np.float32)
    rW = np.asarray(inputs["rW"], dtype=np.float32)
    rb = np.asarray(inputs["rb"], dtype=np.float32)
    W1 = np.asarray(inputs["W1"], dtype=np.float32)
    b1 = np.asarray(inputs["b1"], dtype=np.float32)
    W2 = np.asarray(inputs["W2"], dtype=np.float32)
    b2 = np.asarray(inputs["b2"], dtype=np.float32)
    sW1 = np.asarray(inputs["sW1"], dtype=np.float32)
    sb1 = np.asarray(inputs["sb1"], dtype=np.float32)
    sW2 = np.asarray(inputs["sW2"], dtype=np.float32)
    sb2 = np.asarray(inputs["sb2"], dtype=np.float32)

    B, L, _ = x.shape
    xf = np.ascontiguousarray(x.reshape(-1, D))
    idx, probs = _route(xf, rW, rb)

    tok = []
    prb = []
    for e in range(E):
        sel = idx == e  # (T, K)
        rows = np.nonzero(sel.any(axis=1))[0]
        p = np.where(sel[rows, 0], probs[rows, 0], probs[rows, 1])
        tok.append(rows)
        prb.append(p.astype(np.float32))
    C = max(128, max((len(r) + 127) // 128 * 128 for r in tok))
    CR = C // 128

    nc = _build_nc(C)

    xT_full = np.ascontiguousarray(xf.T).astype(BF)
    in_maps = []
    for core in range(NCORES):
        s, q = core // 4, core % 4
        n_e = len(tok[core])
        xe = np.zeros((D, C), dtype=BF)
        xe[:, :n_e] = xf[tok[core]].T.astype(BF)
        g = np.zeros((CR, 128), dtype=np.float32)
        g.reshape(-1)[:n_e] = prb[core]
        in_maps.append({
            "xeT": xe,
            "g": np.ascontiguousarray(g.T),
            "W1e": np.ascontiguousarray(W1[core]).astype(BF),
            "W2e": np.ascontiguousarray(W2[core]).astype(BF),
            "b1e": np.ascontiguousarray(b1[core].reshape(NHB, 128).T),
            "xT": xT_full,
            "sW1q": np.ascontiguousarray(sW1[s][:, q * HQ:(q + 1) * HQ]).astype(BF),
            "sW2q": np.ascontiguousarray(0.5 * sW2[s][q * HQ:(q + 1) * HQ, :]).astype(BF),
            "sb1q": np.ascontiguousarray(
                sb1[s][q * HQ:(q + 1) * HQ].reshape(8, 128).T),
        })

    if trace:
        _install_ntff_hook()
    res = run_bass_kernel_spmd(nc, in_maps, list(range(NCORES)), trace=trace)

    out = np.zeros((T, D), dtype=np.float32)
    for core in range(NCORES):
        out += res.results[core]["ys"].astype(np.float32)
    out += 0.5 * (sb2[0] + sb2[1])[None, :]
    for e in range(E):
        n_e = len(tok[e])
        out[tok[e]] += res.results[e]["yr"][:n_e].astype(np.float32)
        out[tok[e]] += prb[e][:, None] * b2[e][None, :]
    return out.reshape(B, L, D).astype(np.float32), res


def kernel(**inputs):
    out, _ = _run(inputs, trace=False)
    return out


# revision 31
# speedup vs baseline: 1.0012x; 1.0012x over previous
"""MoE feed-forward kernel for 8 Trainium2 NeuronCores.

Strategy (v2, bf16):
  - Router (tiny: x @ rW, top-2, softmax) runs on host in numpy.
  - Expert-parallel: core e owns routed expert e. Host gathers the tokens
    routed to expert e (padded to the global max capacity C), ships them
    pre-transposed as (D, C) in bf16; the device runs gelu(x@W1+b1) @ W2
    with the per-token gate weight folded in on-chip. Host scatter-adds.
  - Shared experts: sharded (expert s = core//4, hidden-quarter q = core%4).
    Each core computes its quarter of one shared expert over all tokens;
    host sums the 8 bf16 partials (0.5 mean factor folded into sW2).
  - All matmul operands bf16 (halves SBUF read pressure vs fp32r and all
    DMA traffic; rel-err ~3.4e-3, well under the 2e-2 gate).
  - Both phases software-pipeline GEMM1/GEMM2 one block apart so the PE
    never waits on the activation engine.
  - Routed GEMM2 accumulates K=4096 in 6 standing PSUM banks per
    384-token group (3 tok rows x 2 d-halves) - no DVE adds at all.
  - W2 (8.4MB bf16) is fully SBUF-resident, prefetched in the shared
    phase; W1 streams per 128-hidden block on two queues.
"""

import sys
import types

import numpy as np
import ml_dtypes

sys.path.insert(0, "/opt/trn_rl_repo")

import concourse.bass as bass  # noqa: E402
import concourse.mybir as mybir  # noqa: E402
import concourse.tile as tile  # noqa: E402
from concourse import bacc  # noqa: E402
from concourse.bass_utils import run_bass_kernel_spmd  # noqa: E402

F32 = mybir.dt.float32
BF16 = mybir.dt.bfloat16
GELU = mybir.ActivationFunctionType.Gelu

D = 1024      # d_model
H = 4096      # expert hidden
HQ = 1024     # shared-expert hidden slice per core (H / 4)
T = 4096      # tokens (2 * 2048)
E = 8         # routed experts
TOP_K = 2
NCORES = 8
NHB = H // 128  # 32 hidden blocks in routed phase

BF = ml_dtypes.bfloat16


def _install_ntff_hook():
    """Shim for the missing antenv.axon_hooks so trace=True can profile."""
    try:
        import antenv
        if "antenv.axon_hooks" in sys.modules:
            return
        mod = types.ModuleType("antenv.axon_hooks")
        mod._hook = None
        mod.set_axon_ntff_profile_hook = lambda h: setattr(mod, "_hook", h)
        mod.get_axon_ntff_profile_hook = lambda: mod._hook
        sys.modules["antenv.axon_hooks"] = mod
        antenv.axon_hooks = mod
        sys.path.insert(0, "/root/.axon_site/trn_agent_boot")
        import trn_boot
        hook = trn_boot._ntff_profile_via_ctypes("/opt/axon/libaxon_pjrt.so")
        mod.set_axon_ntff_profile_hook(hook)
    except Exception:
        pass


def _tok_groups(CR):
    """Split CR token rows into groups of <=3 rows (6 PSUM banks each)."""
    groups = []
    r = 0
    while r < CR:
        n = min(3, CR - r)
        groups.append((r, n))
        r += n
    return groups


_NC_CACHE = {}


def _build_nc(C):
    if C in _NC_CACHE:
        return _NC_CACHE[C]
    CR = C // 128
    tgs = _tok_groups(CR)

    nc = bacc.Bacc("TRN2", target_bir_lowering=False, debug=False,
                   enable_asserts=True, num_devices=NCORES)

    xeT = nc.dram_tensor("xeT", (D, C), BF16, kind="ExternalInput")
    # small per-partition tensors are uploaded pre-transposed ([128, n],
    # partition-major) so their DMAs are contiguous lines, not 4B gathers
    g_d = nc.dram_tensor("g", (128, CR), F32, kind="ExternalInput")
    W1e = nc.dram_tensor("W1e", (D, H), BF16, kind="ExternalInput")
    W2e = nc.dram_tensor("W2e", (H, D), BF16, kind="ExternalInput")
    b1e = nc.dram_tensor("b1e", (128, NHB), F32, kind="ExternalInput")
    xT = nc.dram_tensor("xT", (D, T), BF16, kind="ExternalInput")
    sW1q = nc.dram_tensor("sW1q", (D, HQ), BF16, kind="ExternalInput")
    sW2q = nc.dram_tensor("sW2q", (HQ, D), BF16, kind="ExternalInput")
    sb1q = nc.dram_tensor("sb1q", (128, 8), F32, kind="ExternalInput")
    yr = nc.dram_tensor("yr", (C, D), BF16, kind="ExternalOutput")
    ys = nc.dram_tensor("ys", (T, D), BF16, kind="ExternalOutput")

    xer = xeT.ap().rearrange("(a p) t -> p a t", p=128)
    W1r = W1e.ap().rearrange("(a p) h -> p a h", p=128)
    W2r = W2e.ap().rearrange("(a p) d -> p a d", p=128)
    sw1r = sW1q.ap().rearrange("(a p) h -> p a h", p=128)
    sw2r = sW2q.ap().rearrange("(a p) d -> p a d", p=128)
    xTr = xT.ap().rearrange("(a p) t -> p a t", p=128)
    ysr = ys.ap().rearrange("(a p) d -> p a d", p=128)
    yrr = yr.ap().rearrange("(a p) d -> p a d", p=128)

    with tile.TileContext(nc) as tc:
        # long-lived pool: routed-phase tensors prefetched during phase S
        with tc.tile_pool(name="pre", bufs=1) as pre:
          w2f = pre.tile([128, NHB, D], BF16)     # full W2, 64KB/part
          xe = pre.tile([128, 8, C], BF16)        # routed tokens
          gt = pre.tile([128, CR], F32)
          b1t = pre.tile([128, NHB], F32)
          wus = pre.tile([128, 128], BF16)        # PE warmup operands
          wum = pre.tile([128, 512], BF16)

          # ---------------- phase S: shared-expert slice over all tokens ----
          with tc.tile_pool(name="swp", bufs=1) as swp, \
             tc.tile_pool(name="sxp", bufs=3) as sxp, \
             tc.tile_pool(name="shp", bufs=26) as shp, \
             tc.tile_pool(name="syp", bufs=3) as syp, \
             tc.tile_pool(name="sph", bufs=2, space="PSUM") as sph, \
             tc.tile_pool(name="spy", bufs=4, space="PSUM") as spy:
            sw1 = swp.tile([128, 8, HQ], BF16)
            sw2 = swp.tile([128, 8, D], BF16)
            sb1t = swp.tile([128, 8], F32)

            # Startup critical path: first GEMM chain needs sw1 + xs[0].
            # Only sync/scalar/gpsimd can initiate DMAs; xs[0] leads the
            # gpsimd queue, sw1 splits across sync+scalar (1MB each). Bulk
            # prefetches are deferred (emitted after block 0's activations
            # on the scalar stream) so they can't steal HBM bandwidth from
            # the critical startup loads.
            # interleave sw1 h-chunks across sync/scalar so consecutive
            # h-tiles become ready in near arrival order
            nc.sync.dma_start(sw1[:, :, 0:128], sw1r[:, :, 0:128])
            nc.scalar.dma_start(sw1[:, :, 128:256], sw1r[:, :, 128:256])
            nc.sync.dma_start(sw1[:, :, 256:512], sw1r[:, :, 256:512])
            nc.scalar.dma_start(sw1[:, :, 512:768], sw1r[:, :, 512:768])
            nc.sync.dma_start(sw1[:, :, 768:1024], sw1r[:, :, 768:1024])
            nc.sync.dma_start(sb1t[:], sb1q.ap()[:])
            # sw2 split across both queues behind sw1; with the lag-2
            # pipeline GEMM2 of block 0 doesn't start until ~38us
            nc.sync.dma_start(sw2[:, 0:4, :], sw2r[:, 0:4, :])
            nc.scalar.dma_start(sw2[:, 4:8, :], sw2r[:, 4:8, :])
            # PE warmup: run throwaway matmuls while the critical DMAs land
            # so the tensor engine is out of its low p-state when real work
            # starts. Operands come from DVE memsets (no DMA dependency).
            nc.vector.memset(wus[:], 0)
            nc.vector.memset(wum[:], 0)
            for wi in range(16):
                pw = sph.tile([128, 512], F32, tag="ph")
                nc.tensor.matmul(pw[:], wus[:], wum[:], start=True, stop=True)

            NB = T // 512
            xs_t = [None] * NB
            hts_t = [None] * NB

            def s_g1(cb):
                xs = sxp.tile([128, 8, 512], BF16, tag="xs")
                # split the load so the first d-pair lands early (startup)
                for dp in range(4):
                    nc.gpsimd.dma_start(
                        xs[:, 2 * dp:2 * dp + 2, :],
                        xTr[:, 2 * dp:2 * dp + 2, cb * 512:(cb + 1) * 512])
                xs_t[cb] = xs
                hts = []
                for h in range(8):
                    ph = sph.tile([128, 512], F32, tag="ph")
                    for d in range(8):
                        nc.tensor.matmul(ph[:], sw1[:, d, h * 128:(h + 1) * 128],
                                         xs[:, d, :], start=(d == 0), stop=(d == 7))
                    ht = shp.tile([128, 512], BF16, tag="ht")
                    nc.scalar.activation(ht[:], ph[:], GELU, bias=sb1t[:, h:h + 1])
                    hts.append(ht)
                hts_t[cb] = hts

            def s_g2(cb):
                hts = hts_t[cb]
                for cs in range(4):
                    for dh in range(2):
                        py = spy.tile([128, 512], F32, tag="py")
                        for h in range(8):
                            nc.tensor.matmul(py[:], hts[h][:, cs * 128:(cs + 1) * 128],
                                             sw2[:, h, dh * 512:(dh + 1) * 512],
                                             start=(h == 0), stop=(h == 7))
                        yt = syp.tile([128, 512], BF16, tag="yt")
                        nc.vector.tensor_copy(yt[:], py[:])
                        nc.gpsimd.dma_start(ysr[:, cb * 4 + cs, dh * 512:(dh + 1) * 512], yt[:])

            # software pipeline: G2(cb) sits between G1(cb+1) and G1(cb+2)
            s_g1(0)
            # routed-phase bulk prefetch. The scheduler ignores program
            # order for independent instructions, so gate these big DMAs
            # behind block-0 compute with a real (WAW) dependency: a tiny
            # DVE copy into each destination tile that reads block 0's
            # first h-tile. Without this the 10.7MB of prefetch saturates
            # HBM right at startup and starves the critical loads.
            # lag-2 software pipeline: G2(cb) between G1(cb+2) and G1(cb+3),
            # so block 0's GEMM2 never waits on the sw2 load
            s_g1(1)
            ht10 = hts_t[1][0]
            nc.vector.tensor_copy(xe[:, 0, 0:2], ht10[:, 0:2])
            nc.vector.tensor_copy(w2f[:, 0, 0:2], ht10[:, 0:2])
            nc.vector.tensor_copy(gt[:, 0:1], ht10[:, 0:1])
            nc.vector.tensor_copy(b1t[:, 0:1], ht10[:, 0:1])
            nc.scalar.dma_start(xe[:], xer[:])
            nc.scalar.dma_start(w2f[:], W2r[:])
            nc.sync.dma_start(gt[:], g_d.ap()[:])
            nc.sync.dma_start(b1t[:], b1e.ap()[:])
            s_g1(2)
            s_g2(0)
            for cb in range(3, NB):
                s_g1(cb)
                s_g2(cb - 3 + 1)
            s_g2(NB - 2)
            s_g2(NB - 1)

          # ---------------- phase R: routed expert -------------------------
          with tc.tile_pool(name="rwp", bufs=8) as rwp, \
             tc.tile_pool(name="rhp", bufs=3) as rhp, \
             tc.tile_pool(name="rgp", bufs=3) as rgp, \
             tc.tile_pool(name="rph", bufs=2, space="PSUM") as rph, \
             tc.tile_pool(name="rac", bufs=1, space="PSUM") as rac:
            for gi, (r0, nr) in enumerate(tgs):
                c0, ct = r0 * 128, nr * 128
                accs = [rac.tile([128, 512], F32, tag=f"acc{i}", bufs=1,
                                 name=f"acc{i}") for i in range(2 * nr)]
                ht_prev = None

                def r_g2(hb, ht):
                    for tr in range(nr):
                        for dh in range(2):
                            acc = accs[tr * 2 + dh]
                            nc.tensor.matmul(acc[:], ht[:, tr * 128:(tr + 1) * 128],
                                             w2f[:, hb, dh * 512:(dh + 1) * 512],
                                             start=(hb == 0), stop=(hb == NHB - 1))
                            if hb == NHB - 1:
                                # gate + store now so the epilogue overlaps;
                                # alternate DVE/scalar so the final muls of
                                # the last group run two-wide
                                yg = rgp.tile([128, 512], BF16, tag="yg")
                                crow = r0 + tr
                                if (tr * 2 + dh) % 2 == 0:
                                    nc.vector.tensor_scalar_mul(
                                        yg[:], acc[:], gt[:, crow:crow + 1])
                                else:
                                    nc.scalar.mul(
                                        yg[:], acc[:], gt[:, crow:crow + 1])
                                q = nc.sync if dh == 0 else nc.gpsimd
                                q.dma_start(
                                    yrr[:, crow, dh * 512:(dh + 1) * 512], yg[:])

                for hb in range(NHB):
                    w1t = rwp.tile([128, 8, 128], BF16, tag="w1")
                    q = nc.sync if hb % 2 == 0 else nc.gpsimd
                    q.dma_start(w1t[:], W1r[:, :, hb * 128:(hb + 1) * 128])
                    w1 = w1t[:]
                    ph = rph.tile([128, 512], F32, tag="ph")
                    for d in range(8):
                        nc.tensor.matmul(ph[:, :ct], w1[:, d, :],
                                         xe[:, d, c0:c0 + ct],
                                         start=(d == 0), stop=(d == 7))
                    ht = rhp.tile([128, 512], BF16, tag="ht")
                    nc.scalar.activation(ht[:, :ct], ph[:, :ct], GELU,
                                         bias=b1t[:, hb:hb + 1])
                    if ht_prev is not None:
                        r_g2(hb - 1, ht_prev)
                    ht_prev = ht
                r_g2(NHB - 1, ht_prev)

    nc.compile()
    nc.finalize()
    _NC_CACHE[C] = nc
    return nc


def _route(xf, rW, rb):
    """Host router: replicates jax top_k (ties -> lower index) + softmax."""
    gates = xf @ rW + rb
    idx = np.argsort(-gates, axis=1, kind="stable")[:, :TOP_K]
    vals = np.take_along_axis(gates, idx, axis=1)
    ex = np.exp(vals - vals[:, :1])
    probs = (ex / ex.sum(axis=1, keepdims=True)).astype(np.float32)
    return idx, probs


def _run(inputs, trace=False):
    x = np.asarray(inputs["x"], dtype=np.float32)
    rW = np.asarray(inputs["rW"], dtype=np.float32)
    rb = np.asarray(inputs["rb"], dtype=np.float32)
    W1 = np.asarray(inputs["W1"], dtype=np.float32)
    b1 = np.asarray(inputs["b1"], dtype=np.float32)
    W2 = np.asarray(inputs["W2"], dtype=np.float32)
    b2 = np.asarray(inputs["b2"], dtype=np.float32)
    sW1 = np.asarray(inputs["sW1"], dtype=np.float32)
    sb1 = np.asarray(inputs["sb1"], dtype=np.float32)
    sW2 = np.asarray(inputs["sW2"], dtype=np.float32)
    sb2 = np.asarray(inputs["sb2"], dtype=np.float32)

    B, L, _ = x.shape
    xf = np.ascontiguousarray(x.reshape(-1, D))
    idx, probs = _route(xf, rW, rb)

    tok = []
    prb = []
    for e in range(E):
        sel = idx == e  # (T, K)
        rows = np.nonzero(sel.any(axis=1))[0]
        p = np.where(sel[rows, 0], probs[rows, 0], probs[rows, 1])
        tok.append(rows)
        prb.append(p.astype(np.float32))
    C = max(128, max((len(r) + 127) // 128 * 128 for r in tok))
    CR = C // 128

    nc = _build_nc(C)

    xT_full = np.ascontiguousarray(xf.T).astype(BF)
    in_maps = []
    for core in range(NCORES):
        s, q = core // 4, core % 4
        n_e = len(tok[core])
        xe = np.zeros((D, C), dtype=BF)
        xe[:, :n_e] = xf[tok[core]].T.astype(BF)
        g = np.zeros((CR, 128), dtype=np.float32)
        g.reshape(-1)[:n_e] = prb[core]
        in_maps.append({
            "xeT": xe,
            "g": np.ascontiguousarray(g.T),
            "W1e": np.ascontiguousarray(W1[core]).astype(BF),
            "W2e": np.ascontiguousarray(W2[core]).astype(BF),
            "b1e": np.ascontiguousarray(b1[core].reshape(NHB, 128).T),
            "xT": xT_full,
            "sW1q": np.ascontiguousarray(sW1[s][:, q * HQ:(q + 1) * HQ]).astype(BF),
            "sW2q": np.ascontiguousarray(0.5 * sW2[s][q * HQ:(q + 1) * HQ, :]).astype(BF),
            "sb1q": np.ascontiguousarray(
                sb1[s][q * HQ:(q + 1) * HQ].reshape(8, 128).T),
        })

    if trace:
        _install_ntff_hook()
    res = run_bass_kernel_spmd(nc, in_maps, list(range(NCORES)), trace=trace)

    out = np.zeros((T, D), dtype=np.float32)
    for core in range(NCORES):
        out += res.results[core]["ys"].astype(np.float32)
    out += 0.5 * (sb2[0] + sb2[1])[None, :]
    for e in range(E):
        n_e = len(tok[e])
        out[tok[e]] += res.results[e]["yr"][:n_e].astype(np.float32)
        out[tok[e]] += prb[e][:, None] * b2[e][None, :]
    return out.reshape(B, L, D).astype(np.float32), res


def kernel(**inputs):
    out, _ = _run(inputs, trace=False)
    return out
